# revision 1
# baseline (speedup 1.0000x reference)
"""GAT (2-layer, 8-head) Trainium2 Bass kernel, 8-way node-sharded.

Strategy:
  - Nodes are partitioned into 8 equal contiguous ranges (2500/core, padded
    to NLOC=2560).  Each core owns the incoming edges of its nodes (dst
    sharding) -> all scatter-adds are core-local.
  - Per layer: each core computes xw = x @ W (+bias folded) and attention
    scores a_src/a_dst for its own nodes (fp32r matmuls), packs bf16 table
    rows [feat(512) | a_src(8) | pad], AllGathers the table across cores.
  - Edge phase: edges sorted by dst, grouped into 128-node blocks, padded to
    TPB tiles of 128 edges.  Per block: dma_gather pulls per-edge src rows
    (bf16) from the gathered table; one-hot M / M_T matrices (built on-chip
    with is_equal against iota constants) implement segment ops on the
    TensorEngine:  a_dst broadcast = M_T.T @ a_dst_block,  segment-sum
    s = M.T @ ex,  out_un = M.T @ (feat * ex).  Normalization by 1/s happens
    once per block (softmax denominators cancel per row).
  - Global mean-pool via host-prescaled per-block batch one-hots (fp32
    matmul into a persistent PSUM accumulator), AllReduce, small linear +
    log_softmax on-chip.  Output [64, 10] is identical on every core.
"""
import os
import sys
import tempfile
from contextlib import ExitStack
from dataclasses import dataclass

import numpy as np

sys.path.insert(0, "/opt/trn_rl_repo")

import ml_dtypes  # noqa: E402

import concourse.bass as bass  # noqa: E402
import concourse.tile as tile  # noqa: E402
from concourse import mybir  # noqa: E402
from concourse import library_config  # noqa: E402
from concourse._compat import with_exitstack  # noqa: E402

P = 128
AF = mybir.ActivationFunctionType
ALU = mybir.AluOpType
DT = mybir.dt
BF16 = ml_dtypes.bfloat16


@dataclass(frozen=True)
class GATConfig:
    n: int = 20000
    e: int = 320000
    in_dim: int = 256
    hid: int = 64
    heads: int = 8
    classes: int = 10
    g: int = 64
    ncore: int = 8
    neg_slope: float = 0.2

    @property
    def d(self):
        return self.hid * self.heads          # 512

    @property
    def nper(self):
        return self.n // self.ncore           # 2500

    @property
    def nb(self):
        return (self.nper + P - 1) // P       # 20 node blocks / core

    @property
    def nloc(self):
        return self.nb * P                    # 2560 padded local rows

    @property
    def tblw(self):
        return self.d + P                     # 640 bf16 -> 1280B rows

    @property
    def ct(self):
        return self.in_dim // P               # contraction tiles layer 1

    @property
    def dt_(self):
        return self.d // P                    # d tiles (4)


CFG = GATConfig()


# --------------------------------------------------------------------------
# Host-side preprocessing
# --------------------------------------------------------------------------

def build_host_data(cfg: GATConfig, edge_index: np.ndarray, batch: np.ndarray):
    """Partition + sort edges, build per-core index/one-hot arrays.

    Returns (tpb, per_core dict of arrays, consts dict).
    """
    n, ncore, nper, nb, nloc = cfg.n, cfg.ncore, cfg.nper, cfg.nb, cfg.nloc
    src = np.concatenate([edge_index[0], np.arange(n, dtype=np.int64)])
    dst = np.concatenate([edge_index[1], np.arange(n, dtype=np.int64)])

    core_of = dst // nper
    per_core_edges = []
    maxblk = 0
    for c in range(ncore):
        m = core_of == c
        es, ed = src[m], dst[m] - c * nper
        order = np.argsort(ed, kind="stable")
        es, ed = es[order], ed[order]
        blk = ed // P
        cnts = np.bincount(blk, minlength=nb)
        maxblk = max(maxblk, int(cnts.max()))
        per_core_edges.append((es, ed, cnts))
    tpb = (maxblk + P - 1) // P
    epb = tpb * P                              # edges per block (padded)
    zrow = nloc - 1                            # local + global zero row idx

    cnt_g = np.bincount(batch, minlength=cfg.g).astype(np.float64)
    inv_cnt = 1.0 / np.maximum(cnt_g, 1.0)

    cores = []
    for c in range(ncore):
        es, ed, cnts = per_core_edges[c]
        n_real_c = min(nper, n - c * nper)
        # padded per-block edge arrays
        src_tid = np.full((nb, epb), zrow, dtype=np.int64)   # global table row
        dst_rel = np.full((nb, epb), 255, dtype=np.uint8)
        off = 0
        for b in range(nb):
            k = int(cnts[b])
            sl = slice(off, off + k)
            src_tid[b, :k] = (es[sl] // nper) * nloc + (es[sl] % nper)
            dst_rel[b, :k] = (ed[sl] - b * P).astype(np.uint8)
            off += k
        assert src_tid.max() < 2 ** 15

        # dma_gather wrapped idx: [128, nb*tpb*8] int16
        g_idx = np.zeros((P, nb * epb // 16), dtype=np.int16)
        for b in range(nb):
            w = np.tile(src_tid[b].reshape(-1, 16).T, (8, 1)).astype(np.int16)
            g_idx[:, b * (epb // 16):(b + 1) * (epb // 16)] = w

        # dst_rel column form [128, nb*tpb] uint8: A[p, b*tpb+t] = rel[b, t*128+p]
        drc = dst_rel.reshape(nb, tpb, P).transpose(2, 0, 1).reshape(P, nb * tpb)
        drc = np.ascontiguousarray(drc)
        # dst_rel row form [1, nb*epb] bf16 (broadcast on device via PE)
        drow = dst_rel.reshape(1, nb * epb).astype(BF16)

        # batch one-hot (host-prescaled by 1/cnt), zero for pad rows
        mb = np.zeros((P, nb * cfg.g), dtype=np.float32)
        for b in range(nb):
            for p_ in range(P):
                node = c * nper + b * P + p_
                if b * P + p_ < n_real_c and node < n:
                    mb[p_, b * cfg.g + batch[node]] = inv_cnt[batch[node]]

        cores.append(dict(g_idx=g_idx, drc=drc, drow=drow, mb=mb))

    consts = dict(
        iota_row=np.tile(np.arange(P, dtype=np.uint8).reshape(1, P), (P, 1)),
        iota_col=np.arange(P, dtype=np.uint8).reshape(P, 1),
    )
    return tpb, cores, consts


def build_weight_data(cfg: GATConfig, W1, att_src1, att_dst1, bias1,
                      W2, att_src2, att_dst2, bias2, lin_w, lin_b):
    """Fold attention vectors into block-diagonal matmul weights (float64)."""
    d, h, hid = cfg.d, cfg.heads, cfg.hid

    def ablock(att_s, att_d):
        A = np.zeros((d, 2 * h), dtype=np.float64)
        for hh in range(h):
            A[hh * hid:(hh + 1) * hid, hh] = att_s[hh]
            A[hh * hid:(hh + 1) * hid, h + hh] = att_d[hh]
        return A

    A1 = ablock(att_src1.astype(np.float64), att_dst1.astype(np.float64))
    A2 = ablock(att_src2.astype(np.float64), att_dst2.astype(np.float64))
    W1A = (W1.astype(np.float64) @ A1).astype(np.float32)
    W2A = (W2.astype(np.float64) @ A2).astype(np.float32)
    b1A = (bias1.astype(np.float64) @ A1).astype(np.float32).reshape(1, 2 * h)
    b2A = (bias2.astype(np.float64) @ A2).astype(np.float32).reshape(1, 2 * h)
    return dict(
        w1=W1.astype(np.float32), w1a=W1A, b1=bias1.reshape(1, d).astype(np.float32),
        b1a=b1A,
        w2=W2.astype(BF16), w2a=W2A.astype(BF16),
        b2=bias2.reshape(1, d).astype(BF16), b2a=b2A.astype(BF16),
        lin_w=lin_w.astype(np.float32), lin_b=lin_b.reshape(1, cfg.classes).astype(np.float32),
    )


# --------------------------------------------------------------------------
# Device kernel
# --------------------------------------------------------------------------

@with_exitstack
def gat_tile_kernel(ctx: ExitStack, tc: tile.TileContext, cfg: GATConfig,
                    tpb: int, outs, ins):
    nc = tc.nc
    d, h2, nb, nloc, tblw = cfg.d, 2 * cfg.heads, cfg.nb, cfg.nloc, cfg.tblw
    ct, dt_ = cfg.ct, cfg.dt_
    epb = tpb * P
    slot = epb // 16                    # idx slots per block
    ntbl = cfg.ncore * nloc
    H = cfg.heads

    (o_out,) = outs
    i = ins

    nc.gpsimd.load_library(library_config.mlp)

    # ---------------- persistent pools ----------------
    pc = ctx.enter_context(tc.tile_pool(name="consts", bufs=1))
    dram = ctx.enter_context(tc.tile_pool(name="dram", bufs=1, space="DRAM"))

    def load_const(ap_in, shape, dtype, name):
        t = pc.tile(shape, dtype, tag=name)
        nc.sync.dma_start(t[:], ap_in)
        return t

    # fp32r copies of layer-1 weights (staging tiles scoped + freed)
    stage = tc.tile_pool(name="stage", bufs=2)
    stage_ctx = stage.__enter__()

    def load_f32r(ap_in, shape, name):
        t0 = stage_ctx.tile(shape, DT.float32, tag="stage")
        nc.sync.dma_start(t0[:], ap_in)
        t = pc.tile(shape, DT.float32r, tag=name)
        nc.vector.tensor_copy(t[:], t0[:])
        return t

    xt = [load_f32r(i["x_t"][k * P:(k + 1) * P, :], [P, nloc], f"xt{k}")
          for k in range(ct)]
    w1 = [load_f32r(i["w1"][k * P:(k + 1) * P, :], [P, d], f"w1_{k}") for k in range(ct)]
    w1a = [load_f32r(i["w1a"][k * P:(k + 1) * P, :], [P, h2], f"w1a_{k}") for k in range(ct)]
    b1 = load_f32r(i["b1"][:], [1, d], "b1")
    b1a = load_f32r(i["b1a"][:], [1, h2], "b1a")
    w2 = [load_const(i["w2"][k * P:(k + 1) * P, :], [P, d], DT.bfloat16, f"w2_{k}")
          for k in range(dt_)]
    w2a = [load_const(i["w2a"][k * P:(k + 1) * P, :], [P, h2], DT.bfloat16, f"w2a_{k}")
           for k in range(dt_)]
    b2 = load_const(i["b2"][:], [1, d], DT.bfloat16, "b2")
    b2a = load_const(i["b2a"][:], [1, h2], DT.bfloat16, "b2a")
    lin_w = [load_f32r(i["lin_w"][k * P:(k + 1) * P, :], [P, cfg.classes], f"lw{k}")
             for k in range(dt_)]
    lin_b = load_f32r(i["lin_b"][:], [1, cfg.classes], "lb")
    iota_row = load_const(i["iota_row"][:], [P, P], DT.uint8, "iota_row")
    iota_col = load_const(i["iota_col"][:], [P, 1], DT.uint8, "iota_col")
    drc = load_const(i["drc"][:], [P, nb * tpb], DT.uint8, "drc")
    g_idx = load_const(i["g_idx"][:], [P, nb * slot], DT.int16, "g_idx")
    mbatch = load_const(i["mb"][:], [P, nb * cfg.g], DT.float32, "mb")

    ones_f = stage_ctx.tile([1, P], DT.float32, tag="ones_f")
    nc.vector.memset(ones_f[:], 1.0)
    ones_r = pc.tile([1, P], DT.float32r, tag="ones_r")
    nc.vector.tensor_copy(ones_r[:], ones_f[:])
    ones_bf = pc.tile([1, P], DT.bfloat16, tag="ones_bf")
    nc.vector.memset(ones_bf[:], 1.0)
    ones64_r = pc.tile([1, cfg.g], DT.float32r, tag="ones64_r")
    nc.vector.tensor_copy(ones64_r[:], ones_f[:, :cfg.g])
    id_bf = pc.tile([P, P], DT.bfloat16, tag="id_bf")
    nc.vector.tensor_tensor(out=id_bf[:], in0=iota_row[:],
                            in1=iota_col[:].to_broadcast([P, P]), op=ALU.is_equal)
    iota_colf = pc.tile([P, 1], DT.float32, tag="iota_colf")
    nc.vector.tensor_copy(iota_colf[:], iota_col[:])
    id_f32 = pc.tile([P, P], DT.float32, tag="id_f32")
    nc.vector.tensor_copy(id_f32[:], id_bf[:])
    zrow_bf = pc.tile([1, tblw], DT.bfloat16, tag="zrow")
    nc.vector.memset(zrow_bf[:], 0.0)
    stage.__exit__(None, None, None)

    # a_dst per layer, kept in SBUF (bf16): [128, nb*h]
    adst_bf = pc.tile([P, nb * H], DT.bfloat16, tag="adst1")
    adst2_bf = pc.tile([P, nb * H], DT.bfloat16, tag="adst2")
    # h^T (bf16) for layer-2 matmuls: [128, dt_*nloc]
    hT = pc.tile([P, dt_ * nloc], DT.bfloat16, tag="hT")

    # DRAM tables
    loc_tbl1 = dram.tile([nloc, tblw], DT.bfloat16, tag="ltbl1")
    full_tbl1 = dram.tile([ntbl, tblw], DT.bfloat16, tag="ftbl1",
                          addr_space="Shared")
    loc_tbl2 = dram.tile([nloc, tblw], DT.bfloat16, tag="ltbl2")
    full_tbl2 = dram.tile([ntbl, tblw], DT.bfloat16, tag="ftbl2",
                          addr_space="Shared")

    groups = [list(range(cfg.ncore))]

    # ---------------- phase 1 / 3: node matmuls + table build ----------------
    def node_phase(layer):
        with tc.tile_pool(name=f"np{layer}", bufs=3) as sb, \
             tc.tile_pool(name=f"npp{layer}", bufs=2, space="PSUM") as ps:
            loc_tbl = loc_tbl1 if layer == 1 else loc_tbl2
            adst = adst_bf if layer == 1 else adst2_bf
            for k in range(nb):
                pxw = ps.tile([P, d], DT.float32, tag="pxw")
                pa = ps.tile([P, h2], DT.float32, tag="pa")
                if layer == 1:
                    for c in range(ct):
                        lhs = xt[c][:, k * P:(k + 1) * P]
                        nc.tensor.matmul(pxw[:], lhsT=lhs, rhs=w1[c][:],
                                         start=(c == 0), stop=False)
                        nc.tensor.matmul(pa[:], lhsT=lhs, rhs=w1a[c][:],
                                         start=(c == 0), stop=False)
                    nc.tensor.matmul(pxw[:], lhsT=ones_r[:], rhs=b1[:],
                                     start=False, stop=True)
                    nc.tensor.matmul(pa[:], lhsT=ones_r[:], rhs=b1a[:],
                                     start=False, stop=True)
                else:
                    for c in range(dt_):
                        lhs = hT[:, c * nloc + k * P: c * nloc + (k + 1) * P]
                        nc.tensor.matmul(pxw[:], lhsT=lhs, rhs=w2[c][:],
                                         start=(c == 0), stop=False)
                        nc.tensor.matmul(pa[:], lhsT=lhs, rhs=w2a[c][:],
                                         start=(c == 0), stop=False)
                    nc.tensor.matmul(pxw[:], lhsT=ones_bf[:], rhs=b2[:],
                                     start=False, stop=True)
                    nc.tensor.matmul(pa[:], lhsT=ones_bf[:], rhs=b2a[:],
                                     start=False, stop=True)
                tbl = sb.tile([P, tblw], DT.bfloat16, tag="tbl")
                nc.scalar.copy(tbl[:, 0:d], pxw[:])
                nc.scalar.copy(tbl[:, d:d + H], pa[:, 0:H])
                nc.scalar.copy(tbl[:, d + H:tblw], pa[:, 0:1].to_broadcast([P, tblw - d - H]))
                nc.vector.tensor_copy(adst[:, k * H:(k + 1) * H], pa[:, H:h2])
                nc.sync.dma_start(loc_tbl[k * P:(k + 1) * P, :], tbl[:])
            # zero row (pads point here)
            nc.sync.dma_start(loc_tbl[nloc - 1:nloc, :], zrow_bf[:])

    # ---------------- phase 2 / 4: edge phase ----------------
    def edge_phase(layer):
        full_tbl = full_tbl1 if layer == 1 else full_tbl2
        adst = adst_bf if layer == 1 else adst2_bf
        with tc.tile_pool(name=f"ep{layer}", bufs=2) as gp, \
             tc.tile_pool(name=f"ep2_{layer}", bufs=2) as sb, \
             tc.tile_pool(name=f"mtp{layer}", bufs=4) as mp, \
             tc.tile_pool(name=f"epp{layer}", bufs=2, space="PSUM") as ps, \
             tc.tile_pool(name=f"epq{layer}", bufs=2, space="PSUM") as ps2:
            for b in range(nb):
                gath = gp.tile([P, tpb, tblw], DT.bfloat16, tag="gath")
                nc.gpsimd.dma_gather(
                    gath[:], full_tbl[:], g_idx[:, b * slot:(b + 1) * slot],
                    epb, epb, tblw, single_packet=False)
                bcr = gp.tile([1, epb], DT.bfloat16, tag="bcr")
                nc.sync.dma_start(bcr[:], i["drow"][:, b * epb:(b + 1) * epb])

                # pass A: a_dst broadcast via M_T matmuls; p_s packed alongside
                pblk = ps2.tile([P, (tpb + 1) * H], DT.float32, tag="pblk")
                padst = pblk[:, 0:tpb * H]
                p_s = pblk[:, tpb * H:(tpb + 1) * H]
                for t in range(tpb):
                    pbc = ps2.tile([P, P], DT.float32, tag="pbc")
                    nc.tensor.matmul(pbc[:], lhsT=ones_bf[:],
                                     rhs=bcr[:, t * P:(t + 1) * P],
                                     start=True, stop=True)
                    mt = mp.tile([P, P], DT.bfloat16, tag="mt")
                    nc.any.tensor_tensor(
                        out=mt[:], in0=pbc[:],
                        in1=iota_colf[:].to_broadcast([P, P]), op=ALU.is_equal)
                    nc.tensor.matmul(padst[:, t * H:(t + 1) * H], lhsT=mt[:],
                                     rhs=adst[:, b * H:(b + 1) * H],
                                     start=True, stop=True)

                # e = asrc + adst ; lrelu ; exp
                asrc_f = sb.tile([P, tpb * H], DT.float32, tag="asrc")
                nc.scalar.copy(
                    asrc_f[:].rearrange("p (a b) -> p a b", a=tpb),
                    gath[:, :, d:d + H])
                e_blk = sb.tile([P, tpb * H], DT.float32, tag="eblk")
                nc.vector.tensor_tensor(out=e_blk[:], in0=asrc_f[:], in1=padst,
                                        op=ALU.add)
                e_mul = sb.tile([P, tpb * H], DT.float32, tag="emul")
                nc.vector.tensor_scalar_mul(e_mul[:], e_blk[:], cfg.neg_slope)
                e_lr = sb.tile([P, tpb * H], DT.float32, tag="elr")
                nc.vector.tensor_tensor(out=e_lr[:], in0=e_blk[:], in1=e_mul[:],
                                        op=ALU.max)
                ex_f = sb.tile([P, tpb * H], DT.float32, tag="exf")
                nc.scalar.activation(ex_f[:], e_lr[:], AF.Exp)
                ex_b = sb.tile([P, tpb * H], DT.bfloat16, tag="exb")
                nc.vector.tensor_copy(ex_b[:], ex_f[:])

                # pass B: segment sums
                p_out = ps.tile([P, d], DT.float32, tag="ps_out")
                for t in range(tpb):
                    m = mp.tile([P, P], DT.bfloat16, tag="m")
                    nc.any.tensor_tensor(
                        out=m[:], in0=iota_row[:],
                        in1=drc[:, b * tpb + t:b * tpb + t + 1].to_broadcast([P, P]),
                        op=ALU.is_equal)
                    msg = mp.tile([P, d], DT.bfloat16, tag="msg")
                    nc.any.tensor_tensor(
                        out=msg[:].rearrange("p (a b) -> p a b", a=H),
                        in0=gath[:, t, 0:d].rearrange("p (a b) -> p a b", a=H),
                        in1=ex_b[:, t * H:(t + 1) * H].unsqueeze(2).to_broadcast(
                            [P, H, cfg.hid]),
                        op=ALU.mult)
                    nc.tensor.matmul(p_s, lhsT=m[:], rhs=ex_b[:, t * H:(t + 1) * H],
                                     start=(t == 0), stop=(t == tpb - 1))
                    nc.tensor.matmul(p_out[:], lhsT=m[:], rhs=msg[:],
                                     start=(t == 0), stop=(t == tpb - 1))

                # normalize + elu
                s_g = sb.tile([P, H], DT.float32, tag="sg")
                nc.vector.tensor_scalar_max(s_g[:], p_s, 1e-30)
                rs = sb.tile([P, H], DT.float32, tag="rs")
                nc.vector.reciprocal(rs[:], s_g[:])
                outn = sb.tile([P, d], DT.float32, tag="outn")
                nc.vector.tensor_tensor(
                    out=outn[:].rearrange("p (a b) -> p a b", a=H),
                    in0=p_out[:].rearrange("p (a b) -> p a b", a=H),
                    in1=rs[:].unsqueeze(2).to_broadcast([P, H, cfg.hid]),
                    op=ALU.mult)
                mn = sb.tile([P, d], DT.float32, tag="mn")
                nc.any.tensor_scalar_min(mn[:], outn[:], 0.0)
                ee = sb.tile([P, d], DT.float32, tag="ee")
                nc.scalar.activation(ee[:], mn[:], AF.Exp)
                em1 = sb.tile([P, d], DT.float32, tag="em1")
                nc.any.tensor_scalar_add(em1[:], ee[:], -1.0)
                h_f = sb.tile([P, d], DT.float32, tag="hf")
                nc.vector.tensor_tensor(out=h_f[:], in0=outn[:], in1=em1[:],
                                        op=ALU.max)

                if layer == 1:
                    h_b = sb.tile([P, d], DT.bfloat16, tag="hb")
                    nc.vector.tensor_copy(h_b[:], h_f[:])
                    for c in range(dt_):
                        ptr = ps2.tile([P, P], DT.bfloat16, tag="ptr")
                        nc.tensor.transpose(ptr[:], h_b[:, c * P:(c + 1) * P], id_bf[:])
                        nc.scalar.copy(hT[:, c * nloc + b * P: c * nloc + (b + 1) * P],
                                       ptr[:])
                else:
                    # pooling: per-block psum then accumulate into SBUF
                    p_pb = ps2.tile([P, dt_ * cfg.g], DT.float32, tag="p_pb")
                    for c in range(dt_):
                        nc.tensor.matmul(
                            p_pb[:, c * cfg.g:(c + 1) * cfg.g],
                            lhsT=h_f[:, c * P:(c + 1) * P],
                            rhs=mbatch[:, b * cfg.g:(b + 1) * cfg.g],
                            start=True, stop=True)
                    nc.vector.tensor_tensor(out=pool_acc[:], in0=pool_acc[:],
                                            in1=p_pb[:], op=ALU.add)

    # persistent pooling SBUF accumulator
    pool_acc = pc.tile([P, dt_ * cfg.g], DT.float32, tag="pool_acc")
    nc.vector.memset(pool_acc[:], 0.0)

    def gather_table(loc, full):
        if cfg.ncore == 1 or os.environ.get("GAT_ABLATE", "") == "nocc":
            nc.sync.dma_start(full[:cfg.nloc, :], loc[:])
        else:
            nc.gpsimd.collective_compute(
                "AllGather", ALU.bypass, replica_groups=groups,
                ins=[loc[:].opt()], outs=[full[:].opt()])

    # ---------------- run phases ----------------
    ablate = os.environ.get("GAT_ABLATE", "")
    node_phase(1)
    gather_table(loc_tbl1, full_tbl1)
    if ablate != "noedge":
        edge_phase(1)
    node_phase(2)
    gather_table(loc_tbl2, full_tbl2)
    if ablate != "noedge":
        edge_phase(2)
    if ablate == "noedge":
        # keep hT/adst/pool_acc defined for shape sanity (zeros)
        nc.vector.memset(hT[:, 0:P], 0.0)

    # ---------------- pooling reduce + classifier ----------------
    with tc.tile_pool(name="fin", bufs=1) as sb, \
         tc.tile_pool(name="finp", bufs=1, space="PSUM") as ps:
        pool_g0 = sb.tile([P, dt_ * cfg.g], DT.float32, tag="pool_g0")
        if cfg.ncore == 1 or os.environ.get("GAT_ABLATE", "") == "nocc":
            nc.vector.tensor_copy(pool_g0[:], pool_acc[:])
        else:
            pool_l = dram.tile([P, dt_ * cfg.g], DT.float32, tag="pool_l")
            pool_r = dram.tile([P, dt_ * cfg.g], DT.float32, tag="pool_r")
            nc.sync.dma_start(pool_l[:], pool_acc[:])
            nc.gpsimd.collective_compute(
                "AllReduce", ALU.add, replica_groups=groups,
                ins=[pool_l[:].opt()], outs=[pool_r[:].opt()])
            nc.sync.dma_start(pool_g0[:], pool_r[:])
        pool_g = sb.tile([P, dt_ * cfg.g], DT.float32r, tag="pool_g")
        nc.vector.tensor_copy(pool_g[:], pool_g0[:])

        p_lg = ps.tile([cfg.classes, cfg.g], DT.float32, tag="p_lg")
        for c in range(dt_):
            nc.tensor.matmul(p_lg[:], lhsT=lin_w[c][:],
                             rhs=pool_g[:, c * cfg.g:(c + 1) * cfg.g],
                             start=(c == 0), stop=False)
        nc.tensor.matmul(p_lg[:], lhsT=lin_b[:], rhs=ones64_r[:],
                         start=False, stop=True)
        lg_sb = sb.tile([cfg.classes, cfg.g], DT.float32, tag="lg_sb")
        nc.vector.tensor_copy(lg_sb[:], p_lg[:])
        p_t = ps.tile([cfg.g, cfg.classes], DT.float32, tag="p_t")
        nc.tensor.transpose(p_t[:], lg_sb[:], id_f32[:cfg.classes, :cfg.classes])
        logit = sb.tile([cfg.g, cfg.classes], DT.float32, tag="logit")
        nc.vector.tensor_copy(logit[:], p_t[:])

        rmax = sb.tile([cfg.g, 1], DT.float32, tag="rmax")
        nc.vector.reduce_max(rmax[:], logit[:], axis=mybir.AxisListType.X)
        sh = sb.tile([cfg.g, cfg.classes], DT.float32, tag="sh")
        nc.vector.tensor_scalar(out=sh[:], in0=logit[:], scalar1=rmax[:],
                                scalar2=None, op0=ALU.subtract)
        exps = sb.tile([cfg.g, cfg.classes], DT.float32, tag="exps")
        nc.scalar.activation(exps[:], sh[:], AF.Exp)
        ssum = sb.tile([cfg.g, 1], DT.float32, tag="ssum")
        nc.vector.reduce_sum(ssum[:], exps[:], axis=mybir.AxisListType.X)
        lns = sb.tile([cfg.g, 1], DT.float32, tag="lns")
        nc.scalar.activation(lns[:], ssum[:], AF.Ln)
        res = sb.tile([cfg.g, cfg.classes], DT.float32, tag="res")
        nc.vector.tensor_scalar(out=res[:], in0=sh[:], scalar1=lns[:],
                                scalar2=None, op0=ALU.subtract)
        nc.sync.dma_start(o_out[:], res[:])


# --------------------------------------------------------------------------
# Program build + run
# --------------------------------------------------------------------------

def build_program(cfg: GATConfig, tpb: int):
    from concourse import bacc
    nc = bacc.Bacc("TRN2", target_bir_lowering=False, debug=False,
                   num_devices=cfg.ncore)
    nb, nloc, h2 = cfg.nb, cfg.nloc, 2 * cfg.heads
    epb = tpb * P
    ins = {}

    def inp(name, shape, dt):
        ins[name] = nc.dram_tensor(name, list(shape), dt, kind="ExternalInput").ap()

    inp("x_t", [cfg.in_dim, nloc], DT.float32)
    inp("w1", [cfg.in_dim, cfg.d], DT.float32)
    inp("w1a", [cfg.in_dim, h2], DT.float32)
    inp("b1", [1, cfg.d], DT.float32)
    inp("b1a", [1, h2], DT.float32)
    inp("w2", [cfg.d, cfg.d], DT.bfloat16)
    inp("w2a", [cfg.d, h2], DT.bfloat16)
    inp("b2", [1, cfg.d], DT.bfloat16)
    inp("b2a", [1, h2], DT.bfloat16)
    inp("lin_w", [cfg.d, cfg.classes], DT.float32)
    inp("lin_b", [1, cfg.classes], DT.float32)
    inp("iota_row", [P, P], DT.uint8)
    inp("iota_col", [P, 1], DT.uint8)
    inp("g_idx", [P, nb * epb // 16], DT.int16)
    inp("drc", [P, nb * tpb], DT.uint8)
    inp("drow", [1, nb * epb], DT.bfloat16)
    inp("mb", [P, nb * cfg.g], DT.float32)

    out_ap = nc.dram_tensor("out", [cfg.g, cfg.classes], DT.float32,
                            kind="ExternalOutput").ap()

    with tile.TileContext(nc) as tc:
        gat_tile_kernel(tc, cfg, tpb, [out_ap], ins)
    nc.compile()
    return nc


_CACHE = {}


def _prepare(cfg: GATConfig, inputs):
    key = "prog"
    if key in _CACHE:
        return _CACHE[key]
    edge_index = np.asarray(inputs["edge_index"])
    batch = np.asarray(inputs["batch"])
    tpb, cores, consts = build_host_data(cfg, edge_index, batch)
    nc = build_program(cfg, tpb)
    _CACHE[key] = (nc, tpb, cores, consts)
    return _CACHE[key]


def make_in_maps(cfg: GATConfig, inputs, cores, consts):
    wd = build_weight_data(cfg, inputs["W1"], inputs["att_src1"], inputs["att_dst1"],
                           inputs["bias1"], inputs["W2"], inputs["att_src2"],
                           inputs["att_dst2"], inputs["bias2"], inputs["lin_w"],
                           inputs["lin_b"])
    x = np.asarray(inputs["x"], dtype=np.float32)
    x_t_full = np.ascontiguousarray(x.T)              # [in_dim, n]
    in_maps = []
    for c in range(cfg.ncore):
        xt = np.zeros((cfg.in_dim, cfg.nloc), dtype=np.float32)
        lo = c * cfg.nper
        hi = min(lo + cfg.nper, cfg.n)
        xt[:, :hi - lo] = x_t_full[:, lo:hi]
        m = dict(
            x_t=xt,
            w1=wd["w1"], w1a=wd["w1a"], b1=wd["b1"], b1a=wd["b1a"],
            w2=wd["w2"], w2a=wd["w2a"], b2=wd["b2"], b2a=wd["b2a"],
            lin_w=wd["lin_w"], lin_b=wd["lin_b"],
            iota_row=consts["iota_row"], iota_col=consts["iota_col"],
            g_idx=cores[c]["g_idx"], drc=cores[c]["drc"],
            drow=cores[c]["drow"], mb=cores[c]["mb"],
        )
        in_maps.append(m)
    return in_maps


def run(cfg: GATConfig, inputs, trace=False):
    from concourse.bass_utils import run_bass_kernel_spmd
    nc, tpb, cores, consts = _prepare(cfg, inputs)
    in_maps = make_in_maps(cfg, inputs, cores, consts)
    res = run_bass_kernel_spmd(nc, in_maps, core_ids=list(range(cfg.ncore)),
                               trace=trace)
    return res


def kernel(**inputs) -> np.ndarray:
    res = run(CFG, inputs, trace=False)
    return np.asarray(res.results[0]["out"])

